# revision 19
# baseline (speedup 1.0000x reference)
"""DPLR-SSM layer kernel for Trainium2 (8 NeuronCores, batch-parallel).

Math: the reference recurrence is
    x_t = M x_{t-1} + B_bar u_t,   M = diag(A_bar) + dt * P Q^H   (n=64 complex)
    y_t = Re(C x_t) + D * u_t
M is time-invariant, so we eigendecompose M = V diag(w) V^{-1} on the host
(tiny, n=64) and run the diagonal system
    x'_t = w x'_{t-1} + B_eff u_t,  y_t = Re(C_eff x'_t) + D u_t
with B_eff = V^{-1} B_bar, C_eff = C V.  The complex diagonal scan is made
real by the phase-rotation trick: with w = rho * e^{i*theta},
z_t = e^{-i*theta*t} x'_t obeys  z_t = rho * z_{t-1} + e^{-i*theta*t} b_t,
which is two independent REAL first-order scans (hardware tensor_tensor_scan).

Per-core layout (2 batches of the 16): everything keyed on 128 partitions:
  - scan tiles: partitions = (component, n) = (2, 64); free = time
  - B-projection:  b[(comp,n), t] = sum_d Bc[d,(comp,n)] * uT[d, t]  (PE)
  - C-projection:  y[t, d] = sum_k G[k, t] * W[k, d] over two K=128 groups
    where G1=[c*zr; s*zi], G2=[s*zr; c*zi] fold the inverse rotation into the
    contraction (no elementwise adds needed).
  - D*u via GpSimd elementwise multiply + fused add during PSUM evacuation.
"""

import math

import numpy as np

N = 64
D = 512
BATCH = 16
SEQ = 4096
NCORES = 8
BPC = BATCH // NCORES  # batches per core = 2

_PROG_CACHE = {}

# Set by test harnesses to capture a hardware profile; harmless defaults.
TRACE = False
LAST_RESULTS = None


def _host_precompute(log_neg_real, imag, P_real, P_imag, Q_real, Q_imag,
                     B_real, B_imag, C_real, C_imag, log_dt, D_vec, L):
    """All small-parameter math in float64 on host; returns fp32 device arrays."""
    dt = math.exp(float(np.asarray(log_dt).reshape(-1)[0]))
    Lam = -np.exp(log_neg_real.astype(np.float64)) + 1j * imag.astype(np.float64)
    A_bar = np.exp(Lam * dt)
    B = B_real.astype(np.float64) + 1j * B_imag.astype(np.float64)
    B_bar = ((A_bar - 1.0) / (Lam + 1e-8) * dt)[:, None] * B          # (n, d)
    P = P_real.astype(np.float64) + 1j * P_imag.astype(np.float64)
    Qc = Q_real.astype(np.float64) - 1j * Q_imag.astype(np.float64)
    C = C_real.astype(np.float64) + 1j * C_imag.astype(np.float64)   # (d, n)

    M = np.diag(A_bar) + dt * (P @ Qc.T)
    w, V = np.linalg.eig(M)
    B_eff = np.linalg.solve(V, B_bar)                                 # (n, d)
    C_eff = C @ V                                                     # (d, n)

    rho = np.abs(w)
    theta = np.angle(w)
    t_idx = np.arange(1, L + 1, dtype=np.float64)
    ang = np.outer(theta, t_idx)                                      # (n, L)
    cos_t = np.cos(ang)
    sin_t = np.sin(ang)

    # T1 (128, L): rows 0:64 cos, rows 64:128 sin
    T1 = np.concatenate([cos_t, sin_t], axis=0).astype(np.float32)
    # T2 (128, L): rows 0:64 -sin, rows 64:128 cos (signs arranged so the
    # half-combines are pure adds and G2 needs no fixup)
    T2 = np.concatenate([-sin_t, cos_t], axis=0).astype(np.float32)

    # rho column (128, 1): per-partition scan coefficient
    rhoc = np.concatenate([rho, rho]).astype(np.float32).reshape(128, 1)

    # B weights, lhsT layout: bcomb[p, c*128+m] = Bc[c*128+p, m]
    # where Bc[d, m] with m=comp*64+n: comp0 -> Re(B_eff)[n,d], comp1 -> Im
    Bc = np.concatenate([B_eff.real, B_eff.imag], axis=0).T           # (512, 128)
    bcomb = Bc.reshape(4, 128, 128).transpose(1, 0, 2).reshape(128, 512)
    bcomb = np.ascontiguousarray(bcomb, dtype=np.float32)

    # C-proj weights (K on partitions): W1 rows n: Cr[d,n]; rows 64+n: -Cr[d,n]
    #                                   W2 rows n: -Ci[d,n]; rows 64+n: -Ci[d,n]
    # G1 = T1*z = [c*zr ; s*zi];  G2 = T2*z = [-s*zr ; c*zi]
    # y = sum_n Cr*(c*zr) + (-Cr)*(s*zi) + Ci*(-s*zr) + (-Ci)*(c*zi)
    Cr = C_eff.real.T                                                 # (n, d)
    Ci = C_eff.imag.T
    W1 = np.concatenate([Cr, -Cr], axis=0)                            # (128, 512)
    W2 = np.concatenate([Ci, -Ci], axis=0)
    cexp = np.concatenate([W1, W2], axis=1).astype(np.float32)        # (128, 1024)

    # D broadcast (128, 2048): each partition row = [D, D, D, D]
    dbc = np.tile(D_vec.astype(np.float32).reshape(1, D), (128, 4))
    dbc = np.ascontiguousarray(dbc, dtype=np.float32)

    return dict(t1=T1, t2=T2, rhoc=rhoc, bcomb=bcomb, cexp=cexp, dbc=dbc)


def _split_multi_waits(nc, mybir):
    """Walrus codegen only honors a single sync-wait slot on compute
    instruction structs (ACT/TS/TT...).  Move surplus waits onto chained
    EventSemaphore instructions on the same engine right before the op —
    in-order engine execution makes this equivalent."""
    n = 0
    for func in nc.m.functions:
        for blk in func.blocks:
            il = blk.instructions
            i = 0
            while i < len(il):
                inst = il[i]
                si = inst.sync_info
                if (si is not None and si.on_wait and len(si.on_wait) > 1
                        and not isinstance(inst, mybir.InstEventSemaphore)):
                    waits = list(si.on_wait)
                    for w in waits[:-1]:
                        ev = mybir.InstEventSemaphore(
                            name=f"EVW-{n}", ins=[], outs=[])
                        n += 1
                        ev.engine = inst.engine
                        ev.sync_info = mybir.SyncInfo(on_wait=[w],
                                                      on_update=[])
                        il.insert(i, ev)
                        i += 1
                    inst.sync_info = mybir.SyncInfo(on_wait=[waits[-1]],
                                                    on_update=si.on_update)
                i += 1
    return n


def _build_program(L, split_waits=True):
    """Build the SPMD Bass program for one core: u (BPC*L, 512) -> y same."""
    import concourse.bass as bass
    import concourse.mybir as mybir
    import concourse.tile as tile
    from concourse.masks import make_identity

    TROWS = BPC * L            # total time-rows per core (8192)
    NSLAB = TROWS // 512       # 512-row slabs (16)
    FP32 = mybir.dt.float32
    Alu = mybir.AluOpType

    nc = bass.Bass()
    u_d = nc.dram_tensor("u", [TROWS, D], FP32, kind="ExternalInput")
    t1_d = nc.dram_tensor("t1", [128, L], FP32, kind="ExternalInput")
    t2_d = nc.dram_tensor("t2", [128, L], FP32, kind="ExternalInput")
    rhoc_d = nc.dram_tensor("rhoc", [128, 1], FP32, kind="ExternalInput")
    bcomb_d = nc.dram_tensor("bcomb", [128, 512], FP32, kind="ExternalInput")
    cexp_d = nc.dram_tensor("cexp", [128, 1024], FP32, kind="ExternalInput")
    dbc_d = nc.dram_tensor("dbc", [128, 2048], FP32, kind="ExternalInput")
    y_d = nc.dram_tensor("y", [TROWS, D], FP32, kind="ExternalOutput")

    with tile.TileContext(nc) as tc:
        with tc.tile_pool(name="persist", bufs=1) as pp:
            binb = pp.tile([128, TROWS], FP32, tag="binb")   # b then G2
            gbuf = pp.tile([128, TROWS], FP32, tag="gbuf")   # G1
            bcomb_s = pp.tile([128, 512], FP32, tag="bcomb")
            rhoc_s = pp.tile([128, 1], FP32, tag="rhoc")
            ident = pp.tile([128, 128], FP32, tag="ident")
            nc.sync.dma_start(out=bcomb_s, in_=bcomb_d[:, :])
            nc.sync.dma_start(out=rhoc_s, in_=rhoc_d[:, :])
            make_identity(nc, ident)

            with tc.tile_pool(name="ph12", bufs=1) as p12:
                t1s = p12.tile([128, L], FP32, tag="t1s")
                t2s = p12.tile([128, L], FP32, tag="t2s")
                rhot = p12.tile([128, L], FP32, tag="rhot")
                tmpA = p12.tile([128, L], FP32, tag="tmpA")
                tmpB = p12.tile([128, L], FP32, tag="tmpB")
                nc.sync.dma_start(out=t1s, in_=t1_d[:, :])
                nc.sync.dma_start(out=t2s, in_=t2_d[:, :])
                # rho tile: memset 1.0 then per-partition scale (DVE: ACT
                # instructions only support a single sync wait in codegen)
                nc.gpsimd.memset(rhot, 1.0)
                nc.vector.tensor_scalar_mul(rhot, rhot, rhoc_s)

                # ---------- Phase 1: u -> transpose -> B-projection ----------
                with (
                    tc.tile_pool(name="slab1", bufs=2) as ps1,
                    tc.tile_pool(name="psum1", bufs=2, space="PSUM") as pp1,
                    tc.tile_pool(name="psum1b", bufs=2, space="PSUM") as pp1b,
                ):
                    for i in range(NSLAB):
                        u_nat = ps1.tile([128, 2048], FP32, tag="u_nat")
                        src = u_d[i * 512:(i + 1) * 512, :]
                        src = src.rearrange("(j p) d -> p j d", p=128)
                        nc.sync.dma_start(
                            out=u_nat.rearrange("p (j d) -> p j d", j=4),
                            in_=src)
                        uT = ps1.tile([128, 2048], FP32, tag="uT")
                        for c in range(4):
                            pt = pp1.tile([128, 512], FP32, tag="pt")
                            for j in range(4):
                                blk = u_nat[:, j * 512 + c * 128:
                                            j * 512 + (c + 1) * 128]
                                nc.tensor.transpose(
                                    pt[:, j * 128:(j + 1) * 128], blk, ident)
                            # DVE copy: this op needs two sync waits (PE RAW
                            # + slot-reuse WAW); ACT only supports one.
                            nc.vector.tensor_copy(uT[:, c * 512:(c + 1) * 512],
                                                  pt)
                        pb = pp1b.tile([128, 512], FP32, tag="pb")
                        for c in range(4):
                            nc.tensor.matmul(
                                pb, bcomb_s[:, c * 128:(c + 1) * 128],
                                uT[:, c * 512:(c + 1) * 512],
                                start=(c == 0), stop=(c == 3))
                        nc.scalar.copy(binb[:, i * 512:(i + 1) * 512], pb)

                # ---------- Phase 2: rotation + scans + inverse rotation ----
                # All tensor_tensor ops keep identical start partitions
                # (walrus checkSBSameStartPartition); the two cross-half
                # combines use ScalarE copy + SWDGE accumulate-DMA instead.
                for b in range(BPC):
                    sl = slice(b * L, (b + 1) * L)
                    bb = binb[:, sl]
                    ga = gbuf[:, sl]
                    # Pa = T1 * b = [c*br ; s*bi]
                    nc.vector.tensor_mul(tmpA, t1s, bb)
                    # Pb = T2 * b = [-s*br ; c*bi]
                    nc.vector.tensor_mul(tmpB, t2s, bb)
                    # bp_r = c*br + s*bi ; bp_i = c*bi - s*br  -> into binb.
                    # Cross-half moves via partition-offset SBUF->SBUF DMA
                    # (accumulate-DMA wedges the device; TT ops must keep
                    # identical start partitions), then aligned in-place adds.
                    nc.sync.dma_start(out=bb[0:64, :], in_=tmpA[64:128, :])
                    nc.sync.dma_start(out=bb[64:128, :], in_=tmpB[0:64, :])
                    nc.vector.tensor_add(bb[0:64, :], bb[0:64, :],
                                         tmpA[0:64, :])
                    nc.vector.tensor_add(bb[64:128, :], bb[64:128, :],
                                         tmpB[64:128, :])
                    # scans: z = scan(rho, bp) both components at once
                    nc.vector.tensor_tensor_scan(
                        tmpA, rhot, bb, 0.0, Alu.mult, Alu.add)
                    # G1 = T1*z = [c*zr ; s*zi] ; G2 = T2*z = [-s*zr ; c*zi]
                    nc.vector.tensor_mul(ga, t1s, tmpA)
                    nc.vector.tensor_mul(bb, t2s, tmpA)

            # ---------- Phase 3: C-projection + D*u + store ------------------
            with (
                tc.tile_pool(name="ph3", bufs=1) as p3,
                tc.tile_pool(name="slab3", bufs=3) as ps3,
                tc.tile_pool(name="psum3", bufs=4, space="PSUM") as pp3,
            ):
                cexp_s = p3.tile([128, 1024], FP32, tag="cexp")
                dbc_s = p3.tile([128, 2048], FP32, tag="dbc")
                nc.sync.dma_start(out=cexp_s, in_=cexp_d[:, :])
                nc.sync.dma_start(out=dbc_s, in_=dbc_d[:, :])
                for i in range(NSLAB):
                    u_nat = ps3.tile([128, 2048], FP32, tag="u_nat3")
                    src = u_d[i * 512:(i + 1) * 512, :]
                    src = src.rearrange("(j p) d -> p j d", p=128)
                    nc.sync.dma_start(
                        out=u_nat.rearrange("p (j d) -> p j d", j=4),
                        in_=src)
                    du = ps3.tile([128, 2048], FP32, tag="du")
                    nc.gpsimd.tensor_mul(du, u_nat, dbc_s)
                    ysl = ps3.tile([128, 2048], FP32, tag="ysl")
                    for j in range(4):
                        py = pp3.tile([128, 512], FP32, tag="py")
                        off = i * 512 + j * 128
                        nc.tensor.matmul(py, gbuf[:, off:off + 128],
                                         cexp_s[:, 0:512],
                                         start=True, stop=False)
                        nc.tensor.matmul(py, binb[:, off:off + 128],
                                         cexp_s[:, 512:1024],
                                         start=False, stop=True)
                        # y = psum + du  (fused via scalar_tensor_tensor)
                        nc.vector.scalar_tensor_tensor(
                            ysl[:, j * 512:(j + 1) * 512], py, 1.0,
                            du[:, j * 512:(j + 1) * 512],
                            Alu.mult, Alu.add)
                    dst = y_d[i * 512:(i + 1) * 512, :]
                    dst = dst.rearrange("(j p) d -> p j d", p=128)
                    nc.sync.dma_start(
                        out=dst, in_=ysl.rearrange("p (j d) -> p j d", j=4))

    if split_waits:
        _split_multi_waits(nc, mybir)
    return nc


def kernel(**inputs):
    from concourse.bass_utils import run_bass_kernel_spmd

    u = np.ascontiguousarray(inputs["u"], dtype=np.float32)
    L = u.shape[1]
    params = _host_precompute(
        inputs["log_neg_real"], inputs["imag"], inputs["P_real"],
        inputs["P_imag"], inputs["Q_real"], inputs["Q_imag"],
        inputs["B_real"], inputs["B_imag"], inputs["C_real"],
        inputs["C_imag"], inputs["log_dt"], inputs["D"], L)

    if L not in _PROG_CACHE:
        _PROG_CACHE[L] = _build_program(L)
    nc = _PROG_CACHE[L]

    in_maps = []
    for c in range(NCORES):
        shard = np.ascontiguousarray(
            u[c * BPC:(c + 1) * BPC].reshape(BPC * L, u.shape[2]))
        m = {"u": shard}
        m.update(params)
        in_maps.append(m)

    kwargs = {}
    if TRACE:
        kwargs = dict(trace=True, stitch_traces=False)
    res = run_bass_kernel_spmd(nc, in_maps, core_ids=list(range(NCORES)),
                               **kwargs)
    global LAST_RESULTS
    LAST_RESULTS = res
    y = np.empty_like(u)
    for c in range(NCORES):
        y[c * BPC:(c + 1) * BPC] = res.results[c]["y"].reshape(BPC, L, u.shape[2])
    return y
